# revision 11
# baseline (speedup 1.0000x reference)
"""Trainium2 Bass kernel for nn_Attention_82403242541756.

Reference semantics (with the dim-0 chunk bug):
  qkv = inputs @ W_qkv + b_qkv                  # [3, 2048, 3072]
  q, k, v = split(qkv, 3, axis=0)               # batch split! q=batch0, k=batch1, v=batch2
  each chunk [1, 2048, 3072] flat-reinterpreted to (3, 16, 2048, 64) = 48 "heads"
  scoresT softmax (no max needed; |scores| < 2.2), ctx, flat-reinterpret, @ W_out + b_out

Sharding (zero communication): core c takes seq rows [256c, 256c+256) of all 3
batch items. Head g's flat chunk [g*131072, (g+1)*131072) of a batch's [2048*3072]
QKV output aligns exactly with rows [256c, 256c+256) for g in [6c, 6c+6), and the
output-side reinterpret puts head g at rows [128g, 128g+128) of the flattened
[6144, 1024] context, i.e. rows [768c, 768c+768) of the final output per core.
"""

import sys

sys.path.insert(0, "/opt/trn_rl_repo")

import numpy as np
import ml_dtypes

from concourse import bacc, bass, mybir, tile
from concourse.bass_utils import run_bass_kernel_spmd

BF16 = mybir.dt.bfloat16
F32 = mybir.dt.float32
AF = mybir.ActivationFunctionType
ALU = mybir.AluOpType

P = 128
N_CORES = 8
SEQ = 2048
H = 1024
HEADS_PER_CORE = 6
ROWS = 256  # seq rows per core
SCALE = float(H) ** -0.5  # 1/32, folded into the exp activation

_NC_CACHE = {}


def _build():
    nc = bacc.Bacc()

    xt_e = nc.declare_dram_parameter("xt", [P, 8, 768], BF16, isOutput=False)
    wq_e = nc.declare_dram_parameter("wq", [P, 8, 3072], BF16, isOutput=False)
    bq_e = nc.declare_dram_parameter("bq", [P, 3072], F32, isOutput=False)
    wo_e = nc.declare_dram_parameter("wo", [P, 8, 1024], BF16, isOutput=False)
    bo_e = nc.declare_dram_parameter("bo", [P, 8], F32, isOutput=False)
    out_e = nc.declare_dram_parameter("outt", [1024, 768], F32, isOutput=True)

    with tile.TileContext(nc) as tc:
        with (
            tc.tile_pool(name="dram", bufs=1, space="DRAM") as dp,
            tc.tile_pool(name="qk", bufs=4) as qkp,
            tc.tile_pool(name="vex", bufs=2) as vxp,
            tc.tile_pool(name="scps", bufs=2, space="PSUM") as scps_p,
        ):
            # Padded to 128 cols so the bf16 XBAR DMA-transpose readback is legal.
            yq = dp.tile([12288, 128], BF16)
            yk = dp.tile([12288, 128], BF16)
            yv = dp.tile([12288, 64], BF16)

            # ---------------- Phase 1: QKV projection ----------------
            with (
                tc.tile_pool(name="w1", bufs=1) as w1p,
                tc.tile_pool(name="ps1", bufs=4, space="PSUM") as ps1,
                tc.tile_pool(name="yb", bufs=4) as ybp,
            ):
                wq_sb = w1p.tile([P, 8, 3072], BF16)
                for k in range(8):
                    nc.sync.dma_start(wq_sb[:, k, :], wq_e[:, k, :])
                xt_sb = w1p.tile([P, 8, 768], BF16)
                nc.sync.dma_start(xt_sb[:], xt_e[:])
                bq_sb = w1p.tile([P, 3072], F32)
                nc.sync.dma_start(bq_sb[:], bq_e[:])

                yq_v = yq.rearrange("(r j) d -> r j d", j=48)
                yk_v = yk.rearrange("(r j) d -> r j d", j=48)
                yv_v = yv.rearrange("(r j) d -> r (j d)", j=48)

                for b in range(3):
                    for m in range(2):
                        psums = {}
                        for half in range(2):
                            for nb3 in range(3):
                                psums[3 * half + nb3] = ps1.tile(
                                    [P, 512], F32, name=f"yps{3*half+nb3}", tag="yps"
                                )
                            for k in range(8):
                                lhs = xt_sb[:, k, b * 256 + 128 * m : b * 256 + 128 * (m + 1)]
                                for nb3 in range(3):
                                    nb = 3 * half + nb3
                                    nc.tensor.matmul(
                                        psums[nb][:],
                                        lhsT=lhs,
                                        rhs=wq_sb[:, k, 512 * nb : 512 * (nb + 1)],
                                        start=(k == 0),
                                        stop=(k == 7),
                                    )
                        for nb in range(6):
                            if b < 2:
                                # widened [*, *, 128] with zeroed pad cols 64:128 so
                                # the DMA-transpose readback sees defined data
                                ybuf = ybp.tile([P, 8, 128], BF16, tag="ybw")
                                nc.vector.memset(ybuf[:, :, 64:128], 0.0)
                                nc.vector.tensor_tensor(
                                    ybuf[:, :, 0:64],
                                    psums[nb].rearrange("p (j d) -> p j d", d=64),
                                    bq_sb[:, 512 * nb : 512 * (nb + 1)].rearrange(
                                        "p (j d) -> p j d", d=64
                                    ),
                                    ALU.add,
                                )
                                dst = (yq_v if b == 0 else yk_v)[
                                    128 * m : 128 * (m + 1), 8 * nb : 8 * (nb + 1), :
                                ]
                                nc.sync.dma_start(dst, ybuf[:])
                            else:
                                ybuf = ybp.tile([P, 512], BF16, tag="ybn")
                                nc.vector.tensor_tensor(
                                    ybuf[:],
                                    psums[nb][:],
                                    bq_sb[:, 512 * nb : 512 * (nb + 1)],
                                    ALU.add,
                                )
                                nc.sync.dma_start(
                                    yv_v[128 * m : 128 * (m + 1), 512 * nb : 512 * (nb + 1)],
                                    ybuf[:],
                                )

            # ---------------- Phase 2: attention + out-proj ----------------
            with (
                tc.tile_pool(name="w2", bufs=1) as w2p,
                tc.tile_pool(name="expp", bufs=1) as expp,
                tc.tile_pool(name="rs", bufs=2) as rsp,
                tc.tile_pool(name="rbc", bufs=2) as rbcp,
                tc.tile_pool(name="stg", bufs=2) as stgp,
            ):
                wo_sb = w2p.tile([P, 8, 1024], BF16)
                nc.sync.dma_start(wo_sb[:], wo_e[:])
                bo_sb = w2p.tile([P, 8], F32)
                nc.sync.dma_start(bo_sb[:], bo_e[:])
                ctxfT = w2p.tile([P, 8, 768], BF16)

                with (
                    tc.tile_pool(name="ctxps", bufs=1, space="PSUM") as ctxps_p,
                ):
                    for l in range(HEADS_PER_CORE):
                    qT = qkp.tile([P, SEQ], BF16, tag="qk")
                    nc.sync.dma_start(qT[:], yq[SEQ * l : SEQ * (l + 1), :], transpose=True)
                    kT = qkp.tile([P, SEQ], BF16, tag="qk")
                    nc.sync.dma_start(kT[:], yk[SEQ * l : SEQ * (l + 1), :], transpose=True)
                    vx = vxp.tile([P, 16, 65], BF16)
                    nc.vector.memset(vx[:, :, 64:65], 1.0)
                    nc.sync.dma_start(
                        vx[:, :, 0:64],
                        yv[SEQ * l : SEQ * (l + 1), :].rearrange(
                            "(so p) d -> p so d", p=P
                        ),
                    )

                    expT = expp.tile([P, 16, SEQ], BF16)
                    ctxps = ctxps_p.tile([65, SEQ], F32)

                    for tt in range(16):
                        for hh in range(2):
                            sc = scps_p.tile([P, 1024], F32)
                            for s2 in range(2):
                                s0 = 1024 * hh + 512 * s2
                                nc.tensor.matmul(
                                    sc[:, 512 * s2 : 512 * (s2 + 1)],
                                    lhsT=kT[0:64, 128 * tt : 128 * (tt + 1)],
                                    rhs=qT[0:64, s0 : s0 + 512],
                                    start=True,
                                    stop=True,
                                )
                            nc.scalar.activation(
                                expT[:, tt, 1024 * hh : 1024 * (hh + 1)],
                                sc[:],
                                AF.Exp,
                                scale=SCALE,
                            )
                        for ss in range(4):
                            nc.tensor.matmul(
                                ctxps[:, 512 * ss : 512 * (ss + 1)],
                                lhsT=vx[:, tt, :],
                                rhs=expT[:, tt, 512 * ss : 512 * (ss + 1)],
                                start=(tt == 0),
                                stop=(tt == 15),
                            )

                    # softmax denominators: row 64 of ctxps = sum_t exp
                    rs = rsp.tile([P, 16], F32, tag="rs")
                    nc.sync.dma_start(rs[:], ctxps[64:65, :])
                    rr = rsp.tile([P, 16], F32, tag="rs")
                    nc.vector.reciprocal(rr[:], rs[:])
                    rrow = rsp.tile([1, SEQ], F32, tag="rrow")
                    nc.sync.dma_start(rrow[:], rr[:])
                    rbc = rbcp.tile([64, SEQ], F32)
                    nc.sync.dma_start(rbc[:], rrow[0:1, :].to_broadcast([64, SEQ]))

                    ctxv = ctxps.rearrange("p (r t) -> p r t", t=16)
                    rbcv = rbc.rearrange("p (r t) -> p r t", t=16)
                    for kk in range(8):
                        for h2 in range(2):
                            sp = 2 * kk + h2
                            nc.vector.tensor_tensor(
                                ctxfT[64 * h2 : 64 * (h2 + 1), kk, 128 * l : 128 * (l + 1)],
                                ctxv[0:64, :, sp],
                                rbcv[:, :, sp],
                                ALU.mult,
                            )

                for m in range(8):
                    ops = ops_p.tile([P, 768], F32)
                    for k in range(8):
                        nc.tensor.matmul(
                            ops[:, 0:512],
                            lhsT=wo_sb[:, k, 128 * m : 128 * (m + 1)],
                            rhs=ctxfT[:, k, 0:512],
                            start=(k == 0),
                            stop=(k == 7),
                        )
                    for k in range(8):
                        nc.tensor.matmul(
                            ops[:, 512:768],
                            lhsT=wo_sb[:, k, 128 * m : 128 * (m + 1)],
                            rhs=ctxfT[:, k, 512:768],
                            start=(k == 0),
                            stop=(k == 7),
                        )
                    stg = stgp.tile([P, 768], F32)
                    nc.vector.tensor_scalar(
                        stg[:], ops[:], bo_sb[:, m : m + 1], None, ALU.add
                    )
                    nc.sync.dma_start(out_e[128 * m : 128 * (m + 1), :], stg[:])

    nc.finalize()
    return nc


def _get_nc():
    if "nc" not in _NC_CACHE:
        _NC_CACHE["nc"] = _build()
    return _NC_CACHE["nc"]


def kernel(inputs, W_qkv, b_qkv, W_out, b_out, _trace=False, _trace_kwargs=None):
    bf = ml_dtypes.bfloat16
    x = np.asarray(inputs, dtype=np.float32)
    Wq = np.asarray(W_qkv, dtype=np.float32)
    bq = np.asarray(b_qkv, dtype=np.float32)
    Wo = np.asarray(W_out, dtype=np.float32)
    bo = np.asarray(b_out, dtype=np.float32)

    wq_s = np.ascontiguousarray(Wq.reshape(8, P, 3072).transpose(1, 0, 2)).astype(bf)
    wo_s = np.ascontiguousarray(Wo.reshape(8, P, 1024).transpose(1, 0, 2)).astype(bf)
    bq_s = np.ascontiguousarray(np.broadcast_to(bq[None, :], (P, 3072))).astype(
        np.float32
    )
    bo_s = np.ascontiguousarray(bo.reshape(8, P).T).astype(np.float32)

    in_maps = []
    for c in range(N_CORES):
        xc = x[:, ROWS * c : ROWS * (c + 1), :]  # [3, 256, 1024]
        xt = (
            xc.transpose(2, 0, 1)
            .reshape(1024, 768)
            .reshape(8, P, 768)
            .transpose(1, 0, 2)
        )
        in_maps.append(
            {
                "xt": np.ascontiguousarray(xt).astype(bf),
                "wq": wq_s,
                "bq": bq_s,
                "wo": wo_s,
                "bo": bo_s,
            }
        )

    nc = _get_nc()
    kw = {}
    if _trace:
        kw["trace"] = True
        if _trace_kwargs:
            kw.update(_trace_kwargs)
    res = run_bass_kernel_spmd(nc, in_maps, core_ids=list(range(N_CORES)), **kw)
    outs = res.results

    out = np.empty((6144, 1024), dtype=np.float32)
    for c in range(N_CORES):
        out[768 * c : 768 * (c + 1), :] = np.asarray(
            outs[c]["outt"], dtype=np.float32
        ).T
    if _trace:
        kernel.last_result = res
    return out.reshape(3, SEQ, H)
